# revision 20
# baseline (speedup 1.0000x reference)
"""HB-LSTM cell fused Trainium2 kernel, data-parallel over 8 NeuronCores.

Computes, for gate order (f, i, o, u, k):
    pre  = x @ Wx[g].T + bx[g] + h_prev @ Uh[g].T + bh[g]
    f,i,o,u = sigmoid(pre[0..3]);  c = tanh(pre[4])
    kp = u*c + (1-u)*kp_prev
    k  = f*k_prev + i*kp
    h  = o*tanh(k)
Returns (h, k, kp), each [B, H] float32.

Sharding: batch dim B=65536 split across 8 cores (8192 rows each); weight
stacks replicated to every core.

Per-core structure (8 groups of 8 b-tiles of 128 rows):
  - x/h loaded row-major (cast to bf16 in-flight or via DVE copy, see
    LOAD_MODE), then ONE whole-group xbar DMA-transpose per input (>=4KB
    contiguous source) yields all 16 feature-major lhsT tiles of the group.
  - Per b-tile: 5-gate pre-activations accumulate in one [128,1280] PSUM
    tile: bias via K=1 ones-matmul (start), then 12 bf16 matmuls. The tanh
    gate's weights/bias are pre-scaled by 2 so that ONE sigmoid over all
    1280 cols yields the gates (tanh(x) = 2*sigmoid(2x)-1 fixed up on DVE).
  - Elementwise tail entirely in fp16 at group granularity (N=2048 per DVE
    op -> 2x perf mode, amortized op overhead, ~4x less rounding error than
    bf16); k_prev/kp_prev cast to fp16 in the load DMA; outputs stored as
    fp16 (upcast to f32 on host).
"""

import contextlib

import numpy as np

import concourse.bacc as bacc
import concourse.mybir as mybir
from concourse import tile
from concourse.bass_utils import run_bass_kernel_spmd

N_CORES = 8
B = 65536
IN = 256
H = 256
G5 = 5
BL = B // N_CORES          # rows per core
NT = BL // 128             # 64 b-tiles per core
GROUP = 8                  # b-tiles per DMA group
NG = NT // GROUP
DG = G5 * H                # 1280 = all-gate column span
F32 = mybir.dt.float32
BF16 = mybir.dt.bfloat16
F16 = mybir.dt.float16
GDT = BF16                 # GEMM compute dtype (PE prefers bf16)
DT = F16                   # elementwise-tail dtype
AF = mybir.ActivationFunctionType
ALU = mybir.AluOpType

# Bench mode: when set, the main loop runs LOOP_N times inside a hardware
# For_i loop so device time dominates RPC overhead in wall-clock.
LOOP_N = None

# Probe mode for HW decomposition benches: None = full kernel,
# "pe" = input loads + transposes + matmuls only (no ACT/DVE/stores),
# "mm" = matmuls only on static SBUF inputs,
# "lt" = input loads + transposes only (no PE/ACT/DVE/stores).
PROBE = None

# Which engine queue issues the three output stores.
STORE_ENGINE = "act"

# x/h load path: "swdge" = cast-in-DMA (Pool SWDGE, Q7-emitted descriptors);
# "hwdge" = f32 HWDGE loads + DVE cast copies.
LOAD_MODE = "swdge"

# k/kp load path: "swdge" cast-in-DMA, or "hwdge" f32 loads + DVE casts.
KP_MODE = "swdge"

# Stage x and h in one SBUF tile and transpose both with a single xbar call
# (halves transpose fixed costs and removes transpose-vs-transpose
# serialization from the HW deadlock guard).
FUSED_TR = True

# PSUM pool buffer count (3 x [128,1280] f32 = 15KB of the 16KB fits).
PSUM_BUFS = 2

# Issue the hT transpose on the scalar (ACT) HWDGE ring instead of sync.
TR_SPLIT = False

_CACHE = {}


def _build():
    if "nc" in _CACHE:
        return _CACHE["nc"]

    nc = bacc.Bacc("TRN2", target_bir_lowering=False, debug=False,
                   num_devices=N_CORES)

    x_d = nc.dram_tensor("x", [BL, IN], F32, kind="ExternalInput")
    h_d = nc.dram_tensor("h_prev", [BL, H], F32, kind="ExternalInput")
    k_d = nc.dram_tensor("k_prev", [BL, H], F32, kind="ExternalInput")
    kp_d = nc.dram_tensor("kp_prev", [BL, H], F32, kind="ExternalInput")
    wx_d = nc.dram_tensor("Wx", [G5, H, IN], F32, kind="ExternalInput")
    bx_d = nc.dram_tensor("bx", [G5, H], F32, kind="ExternalInput")
    uh_d = nc.dram_tensor("Uh", [G5, H, H], F32, kind="ExternalInput")
    bh_d = nc.dram_tensor("bh", [G5, H], F32, kind="ExternalInput")
    ho_d = nc.dram_tensor("h_out", [BL, H], DT, kind="ExternalOutput")
    ko_d = nc.dram_tensor("k_out", [BL, H], DT, kind="ExternalOutput")
    kpo_d = nc.dram_tensor("kp_out", [BL, H], DT, kind="ExternalOutput")

    with tile.TileContext(nc) as tc:
        with tc.tile_pool(name="const", bufs=1) as cpool:
            # --- weights: fp32 -> bf16 (cast in DMA), i-major via xbar ---
            # WT[(side, c)]: [128 (i-chunk c), 1280 (g,h)] bf16 = matmul rhs
            # tanh-gate (g=4) weights pre-scaled by 2 (sigmoid folding).
            WT = {}
            for side in ("x", "h"):
                for c in range(2):
                    WT[side, c] = cpool.tile([128, DG], GDT,
                                             name=f"WT_{side}{c}", tag=f"WT_{side}{c}")
            with tc.tile_pool(name="wload", bufs=2) as wload:
                for side, w_d in (("x", wx_d), ("h", uh_d)):
                    for g in range(G5):
                        w16 = wload.tile([128, 2, IN], GDT, tag="w16")
                        nc.gpsimd.dma_start(
                            w16[:],
                            w_d.ap()[g].rearrange("(hc p) i -> p hc i", p=128))
                        if g == 4:
                            nc.vector.tensor_scalar_mul(w16[:], w16[:], 2.0)
                        for c in range(2):
                            for hc in range(2):
                                col = g * H + hc * 128
                                nc.sync.dma_start(
                                    WT[side, c][:, col:col + 128],
                                    w16[:, hc, c * 128:(c + 1) * 128],
                                    transpose=True)

            # --- bias row bs16 [1,1280] bf16 (tanh gate scaled by 2) ---
            bs16 = cpool.tile([1, DG], GDT, tag="bs16")
            ones16 = cpool.tile([1, 128], GDT, tag="ones16")
            with tc.tile_pool(name="binit", bufs=1) as bpool:
                bxr = bpool.tile([G5, H], F32, tag="bxr")
                nc.sync.dma_start(bxr[:], bx_d.ap())
                bhr = bpool.tile([G5, H], F32, tag="bhr")
                nc.sync.dma_start(bhr[:], bh_d.ap())
                bsr = bpool.tile([G5, H], F32, tag="bsr")
                nc.vector.tensor_add(bsr[:], bxr[:], bhr[:])
                bsg = bpool.tile([G5, H], GDT, tag="bsg")
                nc.vector.tensor_copy(bsg[:], bsr[:])
                # flatten [5,256] -> one row [1,1280] (partition-major order)
                nc.sync.dma_start(bs16[:], bsg[:])
                # tanh-gate (g=4) bias scaled by 2 (sigmoid folding)
                nc.vector.tensor_scalar_mul(bs16[:, 4 * H:], bs16[:, 4 * H:],
                                            2.0)
                nc.vector.memset(ones16[:], 1.0)

            # --- main loop ---
            # row-major staging: [p, n(tile in group), c, q]
            x_cm = x_d.ap().rearrange("(n p) (c q) -> p n c q", p=128, q=128)
            h_cm = h_d.ap().rearrange("(n p) (c q) -> p n c q", p=128, q=128)
            k_t = k_d.ap().rearrange("(n p) i -> p n i", p=128)
            kp_t = kp_d.ap().rearrange("(n p) i -> p n i", p=128)
            ho_t = ho_d.ap().rearrange("(n p) i -> p n i", p=128)
            ko_t = ko_d.ap().rearrange("(n p) i -> p n i", p=128)
            kpo_t = kpo_d.ap().rearrange("(n p) i -> p n i", p=128)

            loop_cm = (tc.For_i(0, LOOP_N, 1) if LOOP_N
                       else contextlib.nullcontext())
            with tc.tile_pool(name="io", bufs=2) as io, \
                 tc.tile_pool(name="work", bufs=2) as work, \
                 tc.tile_pool(name="psum", bufs=PSUM_BUFS, space="PSUM") as pp, \
                 loop_cm:
                if PROBE == "mm":
                    zT = cpool.tile([128, GROUP, 2, 128], GDT, tag="zT",
                                    name="zT")
                    nc.vector.memset(zT[:], 0.0)
                for gi in range(NG):
                    nsl = slice(gi * GROUP, (gi + 1) * GROUP)
                    if PROBE != "mm":
                        if FUSED_TR:
                            xh16 = io.tile([128, 2, GROUP, 2, 128], GDT,
                                           tag="xh16")
                            x16 = xh16[:, 0]
                            h16 = xh16[:, 1]
                        else:
                            x16t = io.tile([128, GROUP, 2, 128], GDT,
                                           tag="x16")
                            h16t = io.tile([128, GROUP, 2, 128], GDT,
                                           tag="h16")
                            x16 = x16t[:]
                            h16 = h16t[:]
                        if LOAD_MODE == "hwdge":
                            hg = GROUP // 2
                            for hi in range(2):
                                hsl = slice(gi * GROUP + hi * hg,
                                            gi * GROUP + (hi + 1) * hg)
                                dsl = slice(hi * hg, (hi + 1) * hg)
                                x32 = io.tile([128, hg, 2, 128], F32,
                                              tag="x32")
                                h32 = io.tile([128, hg, 2, 128], F32,
                                              tag="h32")
                                nc.sync.dma_start(x32[:], x_cm[:, hsl])
                                nc.scalar.dma_start(h32[:], h_cm[:, hsl])
                                nc.vector.tensor_copy(x16[:, dsl], x32[:])
                                nc.vector.tensor_copy(h16[:, dsl], h32[:])
                        else:
                            nc.gpsimd.dma_start(x16, x_cm[:, nsl])
                            nc.gpsimd.dma_start(h16, h_cm[:, nsl])
                        if PROBE != "pe":
                            kr = io.tile([128, GROUP, H], DT, tag="kr")
                            kpp = io.tile([128, GROUP, H], DT, tag="kpp")
                            if KP_MODE == "hwdge":
                                kr32 = io.tile([128, GROUP, H], F32, tag="kr32")
                                kpp32 = io.tile([128, GROUP, H], F32,
                                                tag="kpp32")
                                nc.sync.dma_start(kr32[:], k_t[:, nsl, :])
                                nc.scalar.dma_start(kpp32[:], kp_t[:, nsl, :])
                                nc.vector.tensor_copy(kr[:], kr32[:])
                                nc.vector.tensor_copy(kpp[:], kpp32[:])
                            else:
                                nc.gpsimd.dma_start(kr[:], k_t[:, nsl, :])
                                nc.gpsimd.dma_start(kpp[:], kp_t[:, nsl, :])
                            kp_o = io.tile([128, GROUP, H], DT, tag="kp_o")
                            k_o = io.tile([128, GROUP, H], DT, tag="k_o")
                            h_o = io.tile([128, GROUP, H], DT, tag="h_o")

                        # whole-group xbar transpose(s):
                        # side[:, j, c, :] = lhsT tile (feature-major)
                        if FUSED_TR:
                            xhT = work.tile([128, 2, GROUP, 2, 128], GDT,
                                            tag="xhT")
                            nc.sync.dma_start(xhT[:], xh16[:], transpose=True)
                            xT = xhT[:, 0]
                            hT = xhT[:, 1]
                        else:
                            xTt = work.tile([128, GROUP, 2, 128], GDT,
                                            tag="xT")
                            hTt = work.tile([128, GROUP, 2, 128], GDT,
                                            tag="hT")
                            nc.sync.dma_start(xTt[:], x16, transpose=True)
                            tr2 = nc.scalar if TR_SPLIT else nc.sync
                            tr2.dma_start(hTt[:], h16, transpose=True)
                            xT = xTt[:]
                            hT = hTt[:]
                    else:
                        xT = zT[:]
                        hT = zT[:]
                    if PROBE == "lt":
                        continue

                    if PROBE != "pe" and PROBE != "mm":
                        gates = work.tile([128, GROUP, DG], DT, tag="gates")

                    for j in range(GROUP):
                        ps = pp.tile([128, DG], F32, tag="ps")
                        for n0 in range(0, DG, 512):
                            n1 = min(n0 + 512, DG)
                            nc.tensor.matmul(ps[:, n0:n1], ones16[:],
                                             bs16[:, n0:n1],
                                             start=True, stop=False)
                        for si, (side, aT) in enumerate((("x", xT), ("h", hT))):
                            for c in range(2):
                                lhsT = aT[:, j, c, :] if aT.shape[1] == GROUP \
                                    else aT[:, j, c]
                                last = si == 1 and c == 1
                                for n0 in range(0, DG, 512):
                                    n1 = min(n0 + 512, DG)
                                    nc.tensor.matmul(
                                        ps[:, n0:n1], lhsT,
                                        WT[side, c][:, n0:n1],
                                        start=False, stop=last)
                        if PROBE in ("pe", "mm"):
                            continue
                        # all 5 gates in one sigmoid (tanh gate pre-scaled)
                        nc.scalar.activation(gates[:, j, :], ps[:], AF.Sigmoid)

                    if PROBE in ("pe", "mm"):
                        continue

                    # ---- group elementwise tail, all bf16, N=GROUP*256 ----
                    f_ = gates[:, :, 0:256]
                    i_ = gates[:, :, 256:512]
                    o_ = gates[:, :, 512:768]
                    u_ = gates[:, :, 768:1024]
                    s4 = gates[:, :, 1024:1280]
                    c2 = work.tile([128, GROUP, H], DT, tag="c2")
                    nc.vector.tensor_scalar(c2[:], s4, 2.0, -1.0,
                                            ALU.mult, ALU.add)
                    d = work.tile([128, GROUP, H], DT, tag="d")
                    nc.vector.tensor_sub(d[:], c2[:], kpp[:])
                    e = work.tile([128, GROUP, H], DT, tag="e")
                    nc.vector.tensor_mul(e[:], u_, d[:])
                    nc.vector.tensor_add(kp_o[:], e[:], kpp[:])
                    m = work.tile([128, GROUP, H], DT, tag="d")
                    nc.vector.tensor_mul(m[:], f_, kr[:])
                    n = work.tile([128, GROUP, H], DT, tag="e")
                    nc.vector.tensor_mul(n[:], i_, kp_o[:])
                    nc.vector.tensor_add(k_o[:], m[:], n[:])
                    tk = work.tile([128, GROUP, H], DT, tag="c2")
                    nc.scalar.activation(tk[:], k_o[:], AF.Tanh)
                    nc.vector.tensor_mul(h_o[:], o_, tk[:])

                    st = {"sync": nc.sync, "act": nc.scalar,
                          "pool": nc.gpsimd}[STORE_ENGINE]
                    st.dma_start(kpo_t[:, nsl, :], kp_o[:])
                    st.dma_start(ko_t[:, nsl, :], k_o[:])
                    nc.scalar.dma_start(ho_t[:, nsl, :], h_o[:])

    nc.compile()
    _CACHE["nc"] = nc
    return nc


def kernel(x, h_prev, k_prev, kp_prev, Wx, bx, Uh, bh):
    x = np.asarray(x, dtype=np.float32)
    h_prev = np.asarray(h_prev, dtype=np.float32)
    k_prev = np.asarray(k_prev, dtype=np.float32)
    kp_prev = np.asarray(kp_prev, dtype=np.float32)
    Wx = np.ascontiguousarray(np.asarray(Wx, dtype=np.float32))
    bx = np.ascontiguousarray(np.asarray(bx, dtype=np.float32))
    Uh = np.ascontiguousarray(np.asarray(Uh, dtype=np.float32))
    bh = np.ascontiguousarray(np.asarray(bh, dtype=np.float32))

    nc = _build()
    in_maps = []
    for c in range(N_CORES):
        sl = slice(c * BL, (c + 1) * BL)
        in_maps.append({
            "x": np.ascontiguousarray(x[sl]),
            "h_prev": np.ascontiguousarray(h_prev[sl]),
            "k_prev": np.ascontiguousarray(k_prev[sl]),
            "kp_prev": np.ascontiguousarray(kp_prev[sl]),
            "Wx": Wx, "bx": bx, "Uh": Uh, "bh": bh,
        })
    res = run_bass_kernel_spmd(nc, in_maps, list(range(N_CORES)))
    h_out = np.concatenate(
        [np.asarray(res.results[c]["h_out"]).astype(np.float32)
         for c in range(N_CORES)], axis=0)
    k_out = np.concatenate(
        [np.asarray(res.results[c]["k_out"]).astype(np.float32)
         for c in range(N_CORES)], axis=0)
    kp_out = np.concatenate(
        [np.asarray(res.results[c]["kp_out"]).astype(np.float32)
         for c in range(N_CORES)], axis=0)
    return (h_out, k_out, kp_out)
